# revision 1
# baseline (speedup 1.0000x reference)
"""Trainium2 Bass kernel for BondMessagePassing (chemprop-style D-MPNN).

Algorithm notes (all host indexing is precomputed from the input graph):
  - Edges sorted by dst ("slots").  Processing position j handles edge
    f(j) = rev[sigma(j)] whose src equals dst[sigma(j)] -- so gather-node
    indices along the stream are sorted, enabling the aggregation gather
    to be expressed with host-built 0/1 expansion matrices (E) and the
    scatter-sum with selection matrices (S), both as PE matmuls.
  - H[rev] at position j is exactly the row for slot j (fused with the
    scatter input stream -- one gathered row stream serves both).
  - Linearity: aggW = segsum(H@Wh, dst), so per-edge rows are shipped
    post-Wh and no node-level matmul is needed in the loop.
  - Computed rows are shipped to the core owning the pair slot via one
    AllToAll per layer.
"""
import sys
sys.path.insert(0, "/opt/trn_rl_repo")
import numpy as np
import ml_dtypes

import concourse.bass as bass
import concourse.mybir as mybir
import concourse.tile as tile
from concourse import bacc

P = 128
NCORES = 8
HID = 128
NODE_F = 128
BOND_F = 16
CPG = 16              # chunks (of 128 slots) per group
NRING = 4             # scatter/A2A rings (split WAW chains)
SPG = CPG * P         # slots per group (2048)
DEPTH = 3

BF16 = ml_dtypes.bfloat16
FP8 = ml_dtypes.float8_e4m3


# ----------------------------------------------------------------------------
# host-side graph preprocessing
# ----------------------------------------------------------------------------

def prep(x, edge_attr, edge_index, rev_edge_index, W_i, b_i, W_h, b_h, W_o, b_o):
    N, E = x.shape[0], edge_attr.shape[0]
    src = np.asarray(edge_index[0], dtype=np.int64)
    dst = np.asarray(edge_index[1], dtype=np.int64)
    rev = np.asarray(rev_edge_index, dtype=np.int64)
    assert np.array_equal(src[rev], dst) and np.array_equal(dst[rev], src)

    sigma = np.argsort(dst, kind="stable")          # slot -> edge
    slot_of = np.empty(E, dtype=np.int64)
    slot_of[sigma] = np.arange(E)
    deg = np.bincount(dst, minlength=N)
    node_ptr = np.concatenate([[0], np.cumsum(deg)])  # node -> first slot

    # core split: contiguous node ranges with ~equal edge counts
    marks = (np.arange(1, NCORES) * E) // NCORES
    nsplit = [0] + [int(np.searchsorted(node_ptr, m)) for m in marks] + [N]

    # per-core bins: consecutive nodes, <=128 nodes, <=SPG slots
    bins = []          # per core: list of (n0, ncount, s0, scount)
    for c in range(NCORES):
        bl, n = [], nsplit[c]
        while n < nsplit[c + 1]:
            n0, s0 = n, node_ptr[n]
            while (n < nsplit[c + 1] and n - n0 < P
                   and node_ptr[n + 1] - s0 <= SPG):
                n += 1
            assert n > n0, f"node {n0} degree {deg[n0]} exceeds {SPG}"
            bl.append((n0, n - n0, int(s0), int(node_ptr[n] - s0)))
        bins.append(bl)
    G = max(len(b) for b in bins)
    for bl in bins:
        while len(bl) < G:
            bl.append((0, 0, 0, 0))

    # stream position t = (g, k*128+p) ; real slots, edges, gather nodes
    # slotmap[c][t] = global slot or -1
    slotmap = np.full((NCORES, G * SPG), -1, dtype=np.int64)
    for c in range(NCORES):
        for g, (n0, nc_, s0, sc) in enumerate(bins[c]):
            if sc:
                slotmap[c, g * SPG: g * SPG + sc] = np.arange(s0, s0 + sc)

    owner_of_slot = np.empty(E, dtype=np.int64)
    for c in range(NCORES):
        lo = node_ptr[nsplit[c]]
        hi = node_ptr[nsplit[c + 1]]
        owner_of_slot[lo:hi] = c

    # A2A bucketing: row computed at (c, position t w/ slot j) is consumed at
    # core owner(tau(j)), tau(j) = slot_of[rev[sigma(j)]]
    tau = slot_of[rev[sigma]]                       # slot -> dest slot
    counters = np.zeros((NRING, NCORES, NCORES), dtype=np.int64)
    qq = [None] * NCORES                            # per core: q per real pos
    cds = [None] * NCORES
    for c in range(NCORES):
        real = np.where(slotmap[c] >= 0)[0]
        ring = (real // SPG) % NRING
        js = slotmap[c][real]
        cd = owner_of_slot[tau[js]]
        q = np.zeros(len(js), dtype=np.int64)
        for r in range(NRING):
            for d in range(NCORES):
                m = (cd == d) & (ring == r)
                q[m] = counters[r, c, d] + np.arange(m.sum())
                counters[r, c, d] += m.sum()
        qq[c], cds[c] = q, cd
    B_pad = int(((counters.max() + 16) + 127) // 128 * 128)

    # send_r row = cd * B_pad + q ; recv row = ring * 8 * B_pad + c * B_pad + q
    sidx = np.zeros((NCORES, G * SPG), dtype=np.int32)
    gidx = np.zeros((NCORES, G * SPG), dtype=np.int32)
    recv_src = np.zeros(E, dtype=np.int64)          # dest slot -> recv row
    for c in range(NCORES):
        real = np.where(slotmap[c] >= 0)[0]
        ring = (real // SPG) % NRING
        js = slotmap[c][real]
        sidx[c, real] = (cds[c] * B_pad + qq[c]).astype(np.int32)
        recv_src[tau[js]] = ring * NCORES * B_pad + c * B_pad + qq[c]
        dummy = np.where(slotmap[c] < 0)[0]
        sidx[c, dummy] = np.int32(c * B_pad + B_pad - 1)
    for c in range(NCORES):
        real = np.where(slotmap[c] >= 0)[0]
        gidx[c, real] = recv_src[slotmap[c][real]].astype(np.int32)

    # S / E matrices (fp8): S[c][g] : [SPG slots, 128 nodes]
    S = np.zeros((NCORES, G, SPG, P), dtype=np.float32)
    for c in range(NCORES):
        for g, (n0, ncnt, s0, sc) in enumerate(bins[c]):
            if not sc:
                continue
            t = np.arange(sc)
            nn = dst[sigma[s0 + t]] - n0
            S[c, g, t, nn] = 1.0

    # eaT stream: edge_attr[f(j)]^T at each position
    eaT = np.zeros((NCORES, BOND_F, G * SPG), dtype=np.float32)
    for c in range(NCORES):
        real = np.where(slotmap[c] >= 0)[0]
        js = slotmap[c][real]
        f = rev[sigma[js]]
        eaT[c][:, real] = edge_attr[f].T

    # node-level arrays (padded per group)
    x_pad = np.zeros((NCORES, G * P, NODE_F), dtype=np.float32)
    for c in range(NCORES):
        for g, (n0, ncnt, _, _) in enumerate(bins[c]):
            if ncnt:
                x_pad[c, g * P: g * P + ncnt] = x[n0:n0 + ncnt]
    W_i_x = W_i[:, :NODE_F]                        # [HID, NODE_F]
    xW_pad = np.einsum("cnf,hf->cnh", x_pad, W_i_x).astype(np.float32)

    meta = dict(N=N, E=E, G=G, B_pad=B_pad, bins=bins)
    percore = []
    for c in range(NCORES):
        percore.append({
            "gidx": gidx[c].reshape(G, CPG, P).transpose(0, 2, 1)
                    .reshape(G * P, CPG).astype(np.int32),
            "sidx": sidx[c].reshape(G, CPG, P).transpose(0, 2, 1)
                    .reshape(G * P, CPG).astype(np.int32),
            "S": S[c].reshape(G, CPG, P, P).transpose(0, 2, 1, 3)
                 .reshape(G * P, CPG * P).astype(FP8),
            "E": S[c].transpose(0, 2, 1).reshape(G * P, SPG).astype(FP8),
            "eaT": eaT[c].astype(BF16),
            "x_pad": x_pad[c].astype(np.float32),
            "xT_pad": x_pad[c].T.copy().astype(BF16),
            "xW_pad": xW_pad[c].astype(BF16),
            "WieT": W_i[:, NODE_F:].T.copy().astype(BF16),
            "WhT": W_h.T.copy().astype(BF16),
            "WoxT": W_o[:, :NODE_F].T.copy().astype(BF16),
            "WoMT": W_o[:, NODE_F:].T.copy().astype(BF16),
            "negI": (-np.eye(P)).astype(BF16),
            "Ident": np.eye(P).astype(np.float32),
            "IdentB": np.eye(P).astype(BF16),
            "b_i": b_i.reshape(P, 1).astype(np.float32),
            "b_h": b_h.reshape(P, 1).astype(np.float32),
            "b_o_row": b_o.reshape(1, P).astype(BF16),
        })
    return meta, percore


# ----------------------------------------------------------------------------
# numpy emulation of the device pipeline (for validation)
# ----------------------------------------------------------------------------

def numpy_pipeline(meta, percore):
    G, B_pad = meta["G"], meta["B_pad"]
    nrows = NCORES * B_pad
    # per core: NRING ring send buffers
    send = [[np.zeros((nrows, HID), np.float32) for _ in range(NRING)]
            for _ in range(NCORES)]
    H0bT = [None] * NCORES

    def a2a(sends):
        recvs = []
        for c in range(NCORES):
            rr = np.zeros((NRING * nrows, HID), np.float32)
            for ring in range(NRING):
                for s in range(NCORES):
                    rr[ring * nrows + s * B_pad: ring * nrows
                       + (s + 1) * B_pad] = \
                        sends[s][ring][c * B_pad:(c + 1) * B_pad]
            recvs.append(rr)
        return recvs

    # phase 0
    for c in range(NCORES):
        pc = percore[c]
        E_m = pc["E"].astype(np.float32).reshape(G, P, SPG)
        xW = pc["xW_pad"].astype(np.float32).reshape(G, P, HID)
        eaT = pc["eaT"].astype(np.float32)
        WieT = pc["WieT"].astype(np.float32)
        h0 = np.zeros((HID, G * SPG), np.float32)
        for g in range(G):
            sl = slice(g * SPG, (g + 1) * SPG)
            h0[:, sl] = xW[g].T @ E_m[g] + WieT.T @ eaT[:, sl]
        h0b = h0 + pc["b_i"]
        H0bT[c] = h0b.astype(BF16).astype(np.float32)
        h1 = np.maximum(H0bT[c], 0.0)
        WhT = pc["WhT"].astype(np.float32)
        rows = (h1.T @ WhT).astype(BF16).astype(np.float32)  # [G*SPG, HID]
        # sidx layout [G*P, CPG]: entry (g*P+p, k) is position g*SPG+k*P+p
        si = pc["sidx"].reshape(G, P, CPG).transpose(0, 2, 1) \
            .reshape(G, SPG)
        rws = rows.reshape(G, SPG, HID)
        for g in range(G):
            send[c][g % NRING][si[g]] = rws[g]

    aggs = [None] * NCORES
    for it in range(1, DEPTH + 1):
        recv = a2a(send)
        last = it == DEPTH
        send = [[np.zeros((nrows, HID), np.float32) for _ in range(NRING)]
                for _ in range(NCORES)]
        for c in range(NCORES):
            pc = percore[c]
            gi = pc["gidx"].reshape(G, P, CPG).transpose(0, 2, 1).reshape(-1)
            rows_all = recv[c][gi].reshape(G, SPG, HID)
            S_m = pc["S"].astype(np.float32).reshape(G, P, CPG, P) \
                .transpose(0, 2, 1, 3).reshape(G, SPG, P)
            aggs[c] = np.einsum("gsp,gsh->gph", S_m, rows_all)
            if last:
                continue
            E_m = pc["E"].astype(np.float32).reshape(G, P, SPG)
            aggb = aggs[c].astype(BF16).astype(np.float32)
            psM = (np.einsum("gph,gpt->ght", aggb, E_m)
                   - rows_all.transpose(0, 2, 1))        # [G, HID, SPG]
            h0 = H0bT[c].reshape(HID, G, SPG).transpose(1, 0, 2)
            tmp = (psM + h0).astype(BF16).astype(np.float32)
            hn = np.maximum(tmp + pc["b_h"][None], 0.0).astype(BF16).astype(
                np.float32)
            rhs = pc["WhT"] if it < DEPTH - 1 else pc["IdentB"]
            outrows = np.einsum(
                "ght,hj->gtj", hn, rhs.astype(np.float32)).astype(
                BF16).astype(np.float32).reshape(G, SPG, HID)
            si = pc["sidx"].reshape(G, P, CPG).transpose(0, 2, 1) \
                .reshape(G, SPG)
            for g in range(G):
                send[c][g % NRING][si[g]] = outrows[g]

    # readout
    outs = []
    for c in range(NCORES):
        pc = percore[c]
        out = np.zeros((G * P, HID), np.float32)
        for g in range(G):
            agg3 = aggs[c][g]
            mask = (agg3.sum(axis=1) == 0.0).astype(np.float32)[:, None]
            x_g = pc["x_pad"][g * P:(g + 1) * P]
            M = agg3 + mask * x_g
            xT = pc["xT_pad"].astype(np.float32)[:, g * P:(g + 1) * P]
            MT = M.T.astype(BF16).astype(np.float32)
            o = (xT.T @ pc["WoxT"].astype(np.float32)
                 + MT.T @ pc["WoMT"].astype(np.float32)
                 + pc["b_o_row"].astype(np.float32))
            out[g * P:(g + 1) * P] = np.maximum(o, 0.0)
        outs.append(out)
    return outs


def assemble(meta, outs):
    N = meta["N"]
    full = np.zeros((N, HID), np.float32)
    for c in range(NCORES):
        for g, (n0, ncnt, _, _) in enumerate(meta["bins"][c]):
            if ncnt:
                full[n0:n0 + ncnt] = outs[c][g * P: g * P + ncnt]
    return full


# ----------------------------------------------------------------------------
# bass kernel
# ----------------------------------------------------------------------------

def build_nc(G, B_pad):
    DT = mybir.dt
    nc = bacc.Bacc("TRN2", target_bir_lowering=False, debug=False,
                   num_devices=NCORES)
    t = {}
    def inp(name, shape, dt):
        t[name] = nc.dram_tensor(name, shape, dt, kind="ExternalInput")
        return t[name]

    inp("gidx", [G * P, CPG], DT.int32)
    inp("sidx", [G * P, CPG], DT.int32)
    inp("S", [G * P, SPG], DT.float8e4)
    inp("E", [G * P, SPG], DT.float8e4)
    inp("eaT", [BOND_F, G * SPG], DT.bfloat16)
    inp("x_pad", [G * P, NODE_F], DT.float32)
    inp("xT_pad", [P, G * P], DT.bfloat16)
    inp("xW_pad", [G * P, HID], DT.bfloat16)
    inp("WieT", [BOND_F, HID], DT.bfloat16)
    inp("WhT", [P, P], DT.bfloat16)
    inp("WoxT", [P, P], DT.bfloat16)
    inp("WoMT", [P, P], DT.bfloat16)
    inp("negI", [P, P], DT.bfloat16)
    inp("Ident", [P, P], DT.float32)
    inp("IdentB", [P, P], DT.bfloat16)
    inp("b_i", [P, 1], DT.float32)
    inp("b_h", [P, 1], DT.float32)
    inp("b_o_row", [1, P], DT.bfloat16)
    out_pad = nc.dram_tensor("out_pad", [G * P, HID], DT.float32,
                             kind="ExternalOutput")

    h0t = nc.dram_tensor("h0t", [P, G * SPG], DT.bfloat16)
    sends = [nc.dram_tensor(f"send{r}", [NCORES * B_pad, HID], DT.bfloat16)
             for r in range(NRING)]
    recv = nc.dram_tensor("recv", [NRING * NCORES * B_pad, HID], DT.bfloat16)

    AF = mybir.ActivationFunctionType
    OP = mybir.AluOpType
    RG = [list(range(NCORES))]

    with tile.TileContext(nc) as tc:
        with (
            tc.tile_pool(name="sb", bufs=2) as sb,
            tc.tile_pool(name="sm", bufs=3) as sm,
            tc.tile_pool(name="cst", bufs=1) as cst,
            tc.tile_pool(name="ps", bufs=2, space="PSUM") as ps,
            tc.tile_pool(name="ps1", bufs=2, space="PSUM") as ps1,
        ):
            # constants resident
            c_wiet = cst.tile([BOND_F, HID], DT.bfloat16)
            nc.sync.dma_start(c_wiet[:], t["WieT"][:])
            c_wht = cst.tile([P, P], DT.bfloat16)
            nc.sync.dma_start(c_wht[:], t["WhT"][:])
            c_woxt = cst.tile([P, P], DT.bfloat16)
            nc.sync.dma_start(c_woxt[:], t["WoxT"][:])
            c_womt = cst.tile([P, P], DT.bfloat16)
            nc.sync.dma_start(c_womt[:], t["WoMT"][:])
            c_negi = cst.tile([P, P], DT.bfloat16)
            nc.sync.dma_start(c_negi[:], t["negI"][:])
            c_id = cst.tile([P, P], DT.float32)
            nc.sync.dma_start(c_id[:], t["Ident"][:])
            c_idb = cst.tile([P, P], DT.bfloat16)
            nc.sync.dma_start(c_idb[:], t["IdentB"][:])
            c_bi = cst.tile([P, 1], DT.float32)
            nc.sync.dma_start(c_bi[:], t["b_i"][:])
            c_bh = cst.tile([P, 1], DT.float32)
            nc.sync.dma_start(c_bh[:], t["b_h"][:])
            c_bo = cst.tile([1, P], DT.bfloat16)
            nc.sync.dma_start(c_bo[:], t["b_o_row"][:])
            c_one = cst.tile([1, P], DT.bfloat16)
            nc.vector.memset(c_one[:], 1.0)
            c_xt = cst.tile([P, G * P], DT.bfloat16)
            nc.sync.dma_start(c_xt[:], t["xT_pad"][:])

            import os as _os
            NOIND = bool(_os.environ.get("KNOB_NOIND"))

            def scatter_group(outrows, sidx_t, g):
                send = sends[g % NRING]
                if NOIND:
                    nc.gpsimd.dma_start(
                        send.ap()[0:SPG, :].rearrange(
                            "(k p) h -> p k h", p=P),
                        outrows[:].rearrange("p (k h) -> p k h", h=HID))
                    return
                for k in range(CPG):
                    nc.gpsimd.indirect_dma_start(
                        out=send[:],
                        out_offset=bass.IndirectOffsetOnAxis(
                            ap=sidx_t[:, k:k + 1], axis=0),
                        in_=outrows[:, k * P:(k + 1) * P],
                        in_offset=None)

            # ---------------- phase 0 ----------------
            for g in range(G):
                e_g = sm.tile([P, SPG], DT.float8e4, tag="E0")
                nc.sync.dma_start(e_g[:], t["E"][g * P:(g + 1) * P, :])
                ea_g = sm.tile([BOND_F, SPG], DT.bfloat16, tag="ea")
                nc.sync.dma_start(ea_g[:], t["eaT"][:, g * SPG:(g + 1) * SPG])
                xw_g = sm.tile([P, HID], DT.bfloat16, tag="xw")
                nc.sync.dma_start(xw_g[:], t["xW_pad"][g * P:(g + 1) * P, :])
                sidx_t = sm.tile([P, CPG], DT.int32, tag="si0")
                nc.sync.dma_start(sidx_t[:], t["sidx"][g * P:(g + 1) * P, :])
                outrows = sb.tile([P, SPG], DT.bfloat16, tag="or0")
                for s in range(SPG // 512):
                    cols = slice(s * 512, (s + 1) * 512)
                    pm = ps.tile([P, 512], DT.float32, tag="pm")
                    nc.tensor.matmul(pm[:], lhsT=xw_g[:], rhs=e_g[:, cols],
                                     start=True, stop=False)
                    nc.tensor.matmul(pm[:], lhsT=c_wiet[:], rhs=ea_g[:, cols],
                                     start=False, stop=True)
                    h0b = sb.tile([P, 512], DT.bfloat16, tag="h0b")
                    nc.vector.tensor_scalar(h0b[:], pm[:], c_bi[:], None,
                                            op0=OP.add)
                    nc.sync.dma_start(
                        h0t[:, g * SPG + s * 512: g * SPG + (s + 1) * 512],
                        h0b[:])
                    h1 = sb.tile([P, 512], DT.bfloat16, tag="h1")
                    nc.scalar.activation(h1[:], h0b[:], AF.Relu)
                    pw = ps1.tile([P, 512], DT.float32, tag="pw")
                    for k in range(4):
                        kc = slice(k * P, (k + 1) * P)
                        nc.tensor.matmul(pw[:, kc], lhsT=h1[:, kc],
                                         rhs=c_wht[:], start=True, stop=True)
                    nc.scalar.activation(outrows[:, cols], pw[:], AF.Copy)
                scatter_group(outrows, sidx_t, g)

            # ---------------- iterations ----------------
            W = NCORES * B_pad
            for it in range(1, DEPTH + 1):
                for r in range(NRING):
                    nc.gpsimd.collective_compute(
                        "AllToAll", OP.bypass, replica_groups=RG,
                        ins=[sends[r][:]],
                        outs=[recv.ap()[r * W:(r + 1) * W, :]])
                last = it == DEPTH
                for g in range(G):
                    gi = sm.tile([P, CPG], DT.int32, tag="gi")
                    nc.sync.dma_start(gi[:], t["gidx"][g * P:(g + 1) * P, :])
                    rows = sb.tile([P, SPG], DT.bfloat16, tag="rows")
                    if NOIND:
                        nc.gpsimd.dma_start(
                            rows[:].rearrange("p (k h) -> p k h", h=HID),
                            recv.ap()[0:SPG, :].rearrange(
                                "(k p) h -> p k h", p=P))
                    else:
                        for k in range(CPG):
                            nc.gpsimd.indirect_dma_start(
                                out=rows[:, k * P:(k + 1) * P],
                                out_offset=None,
                                in_=recv[:],
                                in_offset=bass.IndirectOffsetOnAxis(
                                    ap=gi[:, k:k + 1], axis=0))
                    s_g = sm.tile([P, SPG], DT.float8e4, tag="S")
                    nc.sync.dma_start(s_g[:], t["S"][g * P:(g + 1) * P, :])
                    pa = ps.tile([P, HID], DT.float32, tag="pa")
                    for k in range(CPG):
                        kc = slice(k * P, (k + 1) * P)
                        nc.tensor.matmul(pa[:], lhsT=s_g[:, kc],
                                         rhs=rows[:, kc],
                                         start=(k == 0), stop=(k == CPG - 1))
                    if last:
                        # ---- readout ----
                        agg3 = sb.tile([P, HID], DT.float32, tag="agg3")
                        nc.scalar.activation(agg3[:], pa[:], AF.Copy)
                        rsum = sb.tile([P, 1], DT.float32, tag="rsum")
                        nc.vector.tensor_reduce(rsum[:], agg3[:],
                                                axis=mybir.AxisListType.X,
                                                op=OP.add)
                        mask = sb.tile([P, 1], DT.float32, tag="mask")
                        nc.vector.tensor_scalar(mask[:], rsum[:], 0.0, None,
                                                op0=OP.is_equal)
                        x_g = sb.tile([P, NODE_F], DT.float32, tag="xg")
                        nc.sync.dma_start(x_g[:],
                                          t["x_pad"][g * P:(g + 1) * P, :])
                        mx = sb.tile([P, NODE_F], DT.float32, tag="mx")
                        nc.vector.tensor_scalar(mx[:], x_g[:], mask[:], None,
                                                op0=OP.mult)
                        Mg = sb.tile([P, HID], DT.float32, tag="Mg")
                        nc.vector.tensor_tensor(Mg[:], agg3[:], mx[:],
                                                op=OP.add)
                        pt = ps1.tile([P, P], DT.float32, tag="pw")
                        nc.tensor.transpose(pt[:], Mg[:], c_id[:])
                        MgT = sb.tile([P, P], DT.bfloat16, tag="MgT")
                        nc.scalar.activation(MgT[:], pt[:], AF.Copy)
                        po = ps.tile([P, HID], DT.float32, tag="pm")
                        nc.tensor.matmul(
                            po[:], lhsT=c_xt[:, g * P:(g + 1) * P],
                            rhs=c_woxt[:], start=True, stop=False)
                        nc.tensor.matmul(po[:], lhsT=MgT[:], rhs=c_womt[:],
                                         start=False, stop=False)
                        nc.tensor.matmul(po[:], lhsT=c_one[:], rhs=c_bo[:],
                                         start=False, stop=True)
                        og = sb.tile([P, HID], DT.float32, tag="og")
                        nc.scalar.activation(og[:], po[:], AF.Relu)
                        nc.sync.dma_start(out_pad[g * P:(g + 1) * P, :], og[:])
                        continue

                    aggw = sb.tile([P, HID], DT.bfloat16, tag="aggw")
                    nc.scalar.activation(aggw[:], pa[:], AF.Copy)
                    e_g = sm.tile([P, SPG], DT.float8e4, tag="Ei")
                    nc.sync.dma_start(e_g[:], t["E"][g * P:(g + 1) * P, :])
                    h0b_g = sm.tile([P, SPG], DT.bfloat16, tag="h0g")
                    nc.sync.dma_start(h0b_g[:],
                                      h0t[:, g * SPG:(g + 1) * SPG])
                    sidx_t = sm.tile([P, CPG], DT.int32, tag="si")
                    nc.sync.dma_start(sidx_t[:],
                                      t["sidx"][g * P:(g + 1) * P, :])
                    outrows = sb.tile([P, SPG], DT.bfloat16, tag="ori")
                    for s in range(SPG // 512):
                        cols = slice(s * 512, (s + 1) * 512)
                        pm = ps.tile([P, 512], DT.float32, tag="pm")
                        nc.tensor.matmul(pm[:], lhsT=aggw[:], rhs=e_g[:, cols],
                                         start=True, stop=False)
                        for k in range(4):
                            kk = s * 4 + k
                            nc.tensor.matmul(
                                pm[:, k * P:(k + 1) * P],
                                lhsT=rows[:, kk * P:(kk + 1) * P],
                                rhs=c_negi[:], start=False, stop=(k == 3))
                        tmp = sb.tile([P, 512], DT.bfloat16, tag="tmp")
                        nc.vector.tensor_tensor(tmp[:], pm[:],
                                                h0b_g[:, cols], op=OP.add)
                        hn = sb.tile([P, 512], DT.bfloat16, tag="hn")
                        nc.scalar.activation(hn[:], tmp[:], AF.Relu,
                                             bias=c_bh[:])
                        pw = ps1.tile([P, 512], DT.float32, tag="pw")
                        rhs_w = c_wht if it < DEPTH - 1 else c_idb
                        for k in range(4):
                            kc = slice(k * P, (k + 1) * P)
                            nc.tensor.matmul(pw[:, kc], lhsT=hn[:, kc],
                                             rhs=rhs_w[:],
                                             start=True, stop=True)
                        nc.scalar.activation(outrows[:, cols], pw[:], AF.Copy)
                    scatter_group(outrows, sidx_t, g)

    nc.compile()
    return nc


# ----------------------------------------------------------------------------
# public entry point
# ----------------------------------------------------------------------------



# ----------------------------------------------------------------------------
# PJRT SPMD runner (inlined; based on concourse.bass2jax.run_bass_via_pjrt)
# ----------------------------------------------------------------------------

class SpmdRunner:
    def __init__(self, nc, n_cores):
        import jax
        from jax.sharding import Mesh, PartitionSpec
        from jax.experimental.shard_map import shard_map
        from concourse.bass2jax import (
            _bass_exec_p, partition_id_tensor, install_neuronx_cc_hook)
        install_neuronx_cc_hook()
        self.jax = jax
        self.n_cores = n_cores
        in_names, out_names, out_avals, zero_outs = [], [], [], []
        partition_name = (
            nc.partition_id_tensor.name if nc.partition_id_tensor else None)
        for alloc in nc.m.functions[0].allocations:
            if not isinstance(alloc, mybir.MemoryLocationSet):
                continue
            name = alloc.memorylocations[0].name
            if alloc.kind == "ExternalInput":
                if name != partition_name:
                    in_names.append(name)
            elif alloc.kind == "ExternalOutput":
                out_names.append(name)
                shape = tuple(alloc.tensor_shape)
                dtype = mybir.dt.np(alloc.dtype)
                out_avals.append(jax.core.ShapedArray(shape, dtype))
                zero_outs.append(np.zeros(shape, dtype))
        self.in_names, self.out_names = in_names, out_names
        self.out_avals, self.zero_outs = out_avals, zero_outs
        n_params, n_outs = len(in_names), len(out_avals)
        all_in = list(in_names) + list(out_names)
        if partition_name is not None:
            all_in.append(partition_name)

        def _body(*args):
            operands = list(args)
            if partition_name is not None:
                operands.append(partition_id_tensor())
            return tuple(_bass_exec_p.bind(
                *operands, out_avals=tuple(out_avals),
                in_names=tuple(all_in), out_names=tuple(out_names),
                lowering_input_output_aliases=(),
                sim_require_finite=True, sim_require_nnan=True, nc=nc))

        devices = jax.devices()[:n_cores]
        assert len(devices) == n_cores
        self.mesh = Mesh(np.asarray(devices), ("core",))
        self.PartitionSpec = PartitionSpec
        in_specs = (PartitionSpec("core"),) * (n_params + n_outs)
        out_specs = (PartitionSpec("core"),) * len(out_names)
        self.fn = jax.jit(
            shard_map(_body, mesh=self.mesh, in_specs=in_specs,
                      out_specs=out_specs, check_rep=False),
            keep_unused=True)

    def pack(self, in_maps):
        n = self.n_cores
        concat = [np.concatenate(
            [np.asarray(in_maps[c][name]) for c in range(n)], axis=0)
            for name in self.in_names]
        for z in self.zero_outs:
            concat.append(np.zeros((n * z.shape[0], *z.shape[1:]), z.dtype))
        sharding = self.jax.sharding.NamedSharding(
            self.mesh, self.PartitionSpec("core"))
        return [self.jax.device_put(a, sharding) for a in concat]

    def run(self, packed):
        outs = self.fn(*packed)
        self.jax.block_until_ready(outs)
        return outs

    def unpack(self, outs):
        n = self.n_cores
        return [
            {name: np.asarray(outs[i]).reshape(n, *self.out_avals[i].shape)[c]
             for i, name in enumerate(self.out_names)}
            for c in range(n)]

    def time_exec(self, packed, iters=10, warmup=2):
        import time
        for _ in range(warmup):
            self.jax.block_until_ready(self.fn(*packed))
        t0 = time.perf_counter()
        outs = None
        for _ in range(iters):
            outs = self.fn(*packed)
        self.jax.block_until_ready(outs)
        t1 = time.perf_counter()
        return (t1 - t0) / iters, outs


def kernel(**inputs):
    inputs = {k: np.asarray(v) for k, v in inputs.items()}
    meta, percore = prep(**inputs)
    nc = build_nc(meta["G"], meta["B_pad"])
    r = SpmdRunner(nc, NCORES)
    packed = r.pack(percore)
    outs = r.run(packed)
    res = r.unpack(outs)
    return assemble(meta, [res[c]["out_pad"] for c in range(NCORES)])

